# revision 1
# baseline (speedup 1.0000x reference)
"""Trainium2 Bass kernel: per-row InstanceNorm + Linear(512->512) + ReLU.

Computes, for x [N, 512], W [512, 512], b [512]:
    xn = (x - mean_row) * rsqrt(var_row + 1e-5)      (biased var, per row)
    y  = relu(xn @ W.T + b)

Strategy: data-parallel over rows across 8 NeuronCores. Per core, rows are
processed 128 at a time:
  bn_stats/bn_aggr (DVE) -> rstd (ACT sqrt + DVE recip)
  -> normalize+cast bf16 (DVE tensor_scalar)
  -> 4x PE transpose (contraction dim onto partitions)
  -> ACT psum->sbuf copy (cast bf16)
  -> bias matmul (K=1) + 4x accumulating bf16 matmuls vs host-pretransposed W
  -> ACT ReLU evacuation (fp32) -> DMA out.

DMAs batch BATCH row-tiles per transfer with a row-interleaved layout
(partition p holds rows p*BATCH..p*BATCH+BATCH-1 of the batch) so each
partition is one contiguous DRAM run (efficient descriptors). Row ordering
across partitions is irrelevant: every row is normalized and matmul'd
independently, and stores mirror the load layout.

Measured on 8 axon trn2 cores: HW exec time ~355 us/core (DMA roofline for
the 820 MB of fp32 I/O is ~287 us/core at 358 GB/s), max scale-relative
error ~2e-3 (bf16 matmul).
"""

import os
import sys

import numpy as np

sys.path.insert(0, "/opt/trn_rl_repo")

import ml_dtypes  # noqa: E402

import concourse.bacc as bacc  # noqa: E402
import concourse.bass as bass  # noqa: E402
import concourse.tile as tile  # noqa: E402
from concourse import mybir  # noqa: E402
from concourse.bass_utils import run_bass_kernel_spmd  # noqa: E402

N_CORES = 8
N_FULL = 200000
N_IN = 512
N_OUT = 512
P = 128
KC = N_IN // P  # 4 contraction chunks
BATCH = 7  # row-tiles per DMA transfer
ROWS_PER_CORE = 25088  # 28 batches of 7*128; 8*25088 = 200704 >= 200000
N_PAD = ROWS_PER_CORE * N_CORES

EPS = 1e-5

F32 = mybir.dt.float32
BF16 = mybir.dt.bfloat16

LAST_RUN = None  # BassKernelResults of the most recent run (for test harness)


def build_bass(rows_per_core: int) -> bass.Bass:
    rows_per_batch = P * BATCH
    nbatches = rows_per_core // rows_per_batch
    assert rows_per_core % rows_per_batch == 0

    # Bacc (not raw Bass): TRN2 allows at most one sync wait per instruction;
    # Bacc.compile() splits multi-wait instructions into event-semaphore
    # preludes that walrus accepts.
    nc = bacc.Bacc()
    x_d = nc.declare_dram_parameter("x", [rows_per_core, N_IN], F32, isOutput=False)
    wt_d = nc.declare_dram_parameter("wt", [N_IN, N_OUT], BF16, isOutput=False)
    b_d = nc.declare_dram_parameter("bvec", [1, N_OUT], BF16, isOutput=False)
    ident_d = nc.declare_dram_parameter("ident", [P, P], BF16, isOutput=False)
    ones_d = nc.declare_dram_parameter("ones1", [1, P], BF16, isOutput=False)
    y_d = nc.declare_dram_parameter("y", [rows_per_core, N_OUT], F32, isOutput=True)

    with tile.TileContext(nc) as tc:
        with (
            tc.tile_pool(name="singles", bufs=1) as singles,
            tc.tile_pool(name="xin", bufs=3) as xin_pool,
            tc.tile_pool(name="stats", bufs=6) as stats_pool,
            tc.tile_pool(name="xn", bufs=3) as xn_pool,
            tc.tile_pool(name="xnt", bufs=3) as xnt_pool,
            tc.tile_pool(name="yout", bufs=3) as y_pool,
            tc.tile_pool(name="pst", bufs=2, space="PSUM") as pst_pool,
            tc.tile_pool(name="psy", bufs=2, space="PSUM") as psy_pool,
        ):
            # --- constants (loaded once) ---
            wt_sb = singles.tile([P, KC, N_OUT], BF16)  # wt_sb[p, c, o] = W.T[c*128+p, o]
            nc.sync.dma_start(out=wt_sb, in_=wt_d[:, :].rearrange("(c p) o -> p c o", p=P))
            ident_sb = singles.tile([P, P], BF16)
            nc.sync.dma_start(out=ident_sb, in_=ident_d[:, :])
            ones_sb = singles.tile([1, P], BF16)
            nc.sync.dma_start(out=ones_sb, in_=ones_d[:, :])
            bvec_sb = singles.tile([1, N_OUT], BF16)
            nc.sync.dma_start(out=bvec_sb, in_=b_d[:, :])
            eps_sb = singles.tile([P, 1], F32)
            nc.vector.memset(eps_sb, EPS)

            # batch b, partition p, sub-tile j  <->  row b*BATCH*128 + p*BATCH + j
            x_b = x_d[:, :].rearrange("(b p j) i -> b p j i", p=P, j=BATCH)
            y_b = y_d[:, :].rearrange("(b p j) o -> b p j o", p=P, j=BATCH)

            for bidx in range(nbatches):
                xb = xin_pool.tile([P, BATCH, N_IN], F32)
                nc.sync.dma_start(out=xb, in_=x_b[bidx])
                yb = y_pool.tile([P, BATCH, N_OUT], F32)

                for j in range(BATCH):
                    x_sb = xb[:, j, :]
                    # row stats: mean/var in one DVE pass
                    stats = stats_pool.tile([P, 6], F32)
                    nc.vector.bn_stats(out=stats, in_=x_sb)
                    mv = stats_pool.tile([P, 2], F32)
                    nc.vector.bn_aggr(out=mv, in_=stats)
                    # rstd = 1/sqrt(var + eps)
                    sd = stats_pool.tile([P, 1], F32)
                    nc.scalar.activation(
                        out=sd, in_=mv[:, 1:2],
                        func=mybir.ActivationFunctionType.Sqrt,
                        bias=eps_sb[:, :], scale=1.0,
                    )
                    rstd = stats_pool.tile([P, 1], F32)
                    nc.vector.reciprocal(out=rstd, in_=sd)
                    # xn = (x - mean) * rstd  (DVE, fp32 math, bf16 out)
                    xn = xn_pool.tile([P, N_IN], BF16)
                    nc.vector.tensor_scalar(
                        out=xn, in0=x_sb,
                        scalar1=mv[:, 0:1], scalar2=rstd[:, :],
                        op0=mybir.AluOpType.subtract, op1=mybir.AluOpType.mult,
                    )
                    # transpose xn into [i, r] chunks via PE
                    ps_t = pst_pool.tile([P, N_IN], BF16)
                    for c in range(KC):
                        nc.tensor.transpose(
                            ps_t[:, c * P:(c + 1) * P], xn[:, c * P:(c + 1) * P],
                            ident_sb[:, :],
                        )
                    xnt = xnt_pool.tile([P, N_IN], BF16)
                    nc.scalar.copy(xnt[:, :], ps_t[:, :])
                    # y = bias + xn @ W.T  (5 matmuls accumulating in PSUM)
                    ps_y = psy_pool.tile([P, N_OUT], F32)
                    nc.tensor.matmul(
                        ps_y[:, :], ones_sb[:, :], bvec_sb[:, :], start=True, stop=False
                    )
                    for c in range(KC):
                        nc.tensor.matmul(
                            ps_y[:, :],
                            xnt[:, c * P:(c + 1) * P],
                            wt_sb[:, c, :],
                            start=False,
                            stop=(c == KC - 1),
                        )
                    # relu + evacuate to fp32 SBUF
                    nc.scalar.activation(
                        out=yb[:, j, :], in_=ps_y[:, :],
                        func=mybir.ActivationFunctionType.Relu,
                    )
                nc.sync.dma_start(out=y_b[bidx], in_=yb)
    nc.compile()
    return nc


_BASS_CACHE: dict[int, bass.Bass] = {}


def _get_bass(rows_per_core: int) -> bass.Bass:
    if rows_per_core not in _BASS_CACHE:
        _BASS_CACHE[rows_per_core] = build_bass(rows_per_core)
    return _BASS_CACHE[rows_per_core]


def _run(x_pad: np.ndarray, W: np.ndarray, b: np.ndarray, rows_per_core: int) -> np.ndarray:
    """x_pad: [n_cores*rows_per_core, 512] float32. Returns same-shape output."""
    global LAST_RUN
    nc = _get_bass(rows_per_core)
    wt = np.ascontiguousarray(W.T).astype(ml_dtypes.bfloat16)
    bb = np.ascontiguousarray(b.reshape(1, N_OUT)).astype(ml_dtypes.bfloat16)
    ident = np.eye(P, dtype=ml_dtypes.bfloat16)
    ones1 = np.ones((1, P), dtype=ml_dtypes.bfloat16)
    in_maps = [
        {
            "x": np.ascontiguousarray(x_pad[c * rows_per_core:(c + 1) * rows_per_core]),
            "wt": wt,
            "bvec": bb,
            "ident": ident,
            "ones1": ones1,
        }
        for c in range(N_CORES)
    ]
    trace = bool(os.environ.get("BASS_TRACE"))
    res = run_bass_kernel_spmd(nc, in_maps, list(range(N_CORES)), trace=trace)
    LAST_RUN = res
    return np.concatenate([res.results[c]["y"] for c in range(N_CORES)], axis=0)


def kernel(x: np.ndarray, W: np.ndarray, b: np.ndarray) -> np.ndarray:
    x = np.asarray(x, dtype=np.float32)
    W = np.asarray(W, dtype=np.float32)
    b = np.asarray(b, dtype=np.float32)
    n = x.shape[0]
    x_pad = np.zeros((N_PAD, N_IN), dtype=np.float32)
    x_pad[:n] = x
    y_pad = _run(x_pad, W, b, ROWS_PER_CORE)
    return np.ascontiguousarray(y_pad[:n])

